# revision 17
# baseline (speedup 1.0000x reference)
"""Trainium2 Bass kernel for nn_Attention_69801808495308 (v2).

Softmax-free attention: attn = cos_w*cossim + cov_w*cov/d + var_w*varprod/d is
linear in k-side summaries, so attn @ f_v reassociates into per-head 64x64
matrices (linear-attention trick) - no NxN score matrix is materialized.

Per (group g, head h), with fk/fv/fq the projected features:
  M1 = (fk/||fk||)^T fv_true        [64,64]
  M2 = (fk - mean(fk))^T fv_true    [64,64]   (columns sum to 0 -> q-centering free)
  m3 = kvar^T fv_true               [64]
  out = sum_h U_q[h] @ C[h] + qvar @ C[8]
  where C[h] = B_h^T-derived [128,512] folds the output projection into the
  per-head summary (B_h = [cos_w*M1; (cov_w/d)*M2]) and C[8] rows 0..7 hold
  RW = (var_w/d)*blockdiag(m3) @ woT - the qvar term rides as a 9th "head".

Sharding: 8 cores = (group g in 0..3) x (row half s in 0..1); q and k/v rows
split across the pair. The per-core partial [B^T; RW] (fp16, 139KB) is
finished by a pairwise AllReduce - the only cross-core communication.

v2 schedule (vs v1): all k/v work first -> M-chain -> AllReduce issued as
early as possible -> the whole q-side pipeline (projection, U_q builds, DMA
transposes) hides the collective -> C build -> attention tail starts with a
warm PE (v1 lost ~50us to a post-M dead zone + HAM cold-clock on the tail).

Other v2 changes:
- M-chain emitted transposed (lhsT=fv, rhs=U_k) so B^T is produced directly;
  C_h = matmul(lhsT=B_h^T, rhs=woT rows) needs no B transposes.
- Head sums/sumsq via one grouped bn_stats per tile ([P,8,64]->[P,8,6])
  instead of reduce_sum+square+reduce_sum (saves ~1.5us DVE per tile).
- x tiles loaded f32 via HWDGE (big 1MB DMAs); the f32->fp16 cast happens in
  the LN-centering ACT op. GpSimd stays idle so the collective trigger is
  not queued behind SWDGE work.
- U-tensor builds in pure fp16 (per-token scalars pre-cast) for 2x DVE rate.
- LayerNorm folded: centering as (mean - x); global sign flip cancelled by
  negating w_out on host (beta, b_out asserted 0). 1/sigma absorbed into
  U-tensor builds; cosine term is scale-invariant.
"""
import numpy as np
from contextlib import ExitStack

import concourse.bass as bass
from concourse import bacc
import concourse.tile as tile
import concourse.mybir as mybir
from concourse.bass_utils import run_bass_kernel_spmd
from concourse.masks import make_identity

f32 = mybir.dt.float32
fp16 = mybir.dt.float16
ALU = mybir.AluOpType
ACTF = mybir.ActivationFunctionType
AXX = mybir.AxisListType.X

QG, N, D = 4, 2048, 512
H, HD = 8, 64
P = 128
LN_EPS = 1e-5
TQ, TK = N // 2, N // 2
QT, KT = TQ // P, TK // P
NCORES = 8


def build_kernel(cos_w, cov_w, var_w):
    c_cov = cov_w / HD
    c_var = var_w / HD

    nc = bacc.Bacc("TRN2", target_bir_lowering=False, debug=False,
                   num_devices=NCORES)
    xq = nc.declare_dram_parameter("xq", [TQ, D], f32, isOutput=False)
    xk = nc.declare_dram_parameter("xk", [TK, D], f32, isOutput=False)
    xv = nc.declare_dram_parameter("xv", [TK, D], f32, isOutput=False)
    wgT_d = nc.declare_dram_parameter("wgT", [D, D], f32, isOutput=False)
    woT_d = nc.declare_dram_parameter("woT", [D, D], f32, isOutput=False)
    out_d = nc.declare_dram_parameter("out", [TQ, D], f32, isOutput=True)

    with tile.TileContext(nc) as tc, ExitStack() as ctx:
        cp = ctx.enter_context(tc.tile_pool(name="cp", bufs=1))
        xcp = ctx.enter_context(tc.tile_pool(name="xcp", bufs=4))
        slp = ctx.enter_context(tc.tile_pool(name="slp", bufs=4))
        sp = ctx.enter_context(tc.tile_pool(name="sp", bufs=4))
        uqp = ctx.enter_context(tc.tile_pool(name="uqp", bufs=1))
        evp = ctx.enter_context(tc.tile_pool(name="evp", bufs=3))
        psF = ctx.enter_context(tc.tile_pool(name="psF", bufs=3, space="PSUM"))
        psT = ctx.enter_context(tc.tile_pool(name="psT", bufs=3, space="PSUM"))
        psM = ctx.enter_context(tc.tile_pool(name="psM", bufs=1, space="PSUM"))
        psR = ctx.enter_context(tc.tile_pool(name="psR", bufs=1, space="PSUM"))

        # ---- big HWDGE input loads (f32, 1MB each; casts happen later) ----
        xk_raw = cp.tile([P, KT, D], f32)
        xv_raw = cp.tile([P, KT, D], f32)
        xq_raw = cp.tile([P, QT, D], f32)
        for half in range(2):
            r0, r1, t0 = half * TK // 2, (half + 1) * TK // 2, half * KT // 2
            nc.sync.dma_start(
                xk_raw[:, t0:t0 + KT // 2, :],
                xk[r0:r1, :].rearrange("(t p) d -> p t d", p=P))
            nc.sync.dma_start(
                xv_raw[:, t0:t0 + KT // 2, :],
                xv[r0:r1, :].rearrange("(t p) d -> p t d", p=P))
        # weights via SWDGE cast-load (GpSimd is otherwise idle early)
        wgT_sb = cp.tile([P, 4, D], fp16)
        nc.gpsimd.dma_start(wgT_sb[:], wgT_d[:].rearrange("(c p) n -> p c n", p=P))
        for half in range(2):
            r0, r1, t0 = half * TQ // 2, (half + 1) * TQ // 2, half * QT // 2
            nc.sync.dma_start(
                xq_raw[:, t0:t0 + QT // 2, :],
                xq[r0:r1, :].rearrange("(t p) d -> p t d", p=P))
        woT_sb = cp.tile([P, 4, D], fp16)
        nc.gpsimd.dma_start(woT_sb[:], woT_d[:].rearrange("(c p) n -> p c n", p=P))

        # ---- constants ----
        ident16 = cp.tile([P, P], fp16)
        make_identity(nc, ident16)
        eps_b = cp.tile([P, 1], f32)
        nc.vector.memset(eps_b[:], LN_EPS)
        bdmask = cp.tile([H, 512], f32)
        nc.gpsimd.memset(bdmask[:], 0.0)
        nc.gpsimd.affine_select(
            out=bdmask[:].rearrange("p (b d) -> p b d", b=H),
            in_=bdmask[:].rearrange("p (b d) -> p b d", b=H),
            compare_op=ALU.not_equal, fill=1.0, base=0,
            pattern=[[-1, H], [0, HD]], channel_multiplier=1)

        # ---- persistent state ----
        fk_all = cp.tile([P, KT, D], fp16)
        fv_all = cp.tile([P, KT, D], fp16)
        fq_all = cp.tile([P, QT, D], fp16)
        uk_all = cp.tile([P, KT, H, 2, HD], fp16)
        uq_all = cp.tile([P, QT, 9, 2, HD], fp16)   # block 8 = qvar row + zeros
        st2_k = cp.tile([P, KT, 2], f32)
        st2_v = cp.tile([P, KT, 2], f32)
        st2_q = cp.tile([P, QT, 2], f32)
        ksum = cp.tile([P, KT, H], fp16)
        ksq = cp.tile([P, KT, H], fp16)
        qsum = cp.tile([P, QT, H], fp16)
        qsq = cp.tile([P, QT, H], fp16)
        C_sb = cp.tile([P, 9, D], fp16)
        nc.gpsimd.memset(uq_all[:, :, 8, :, :], 0.0)
        nc.gpsimd.memset(C_sb[:, 8, :], 0.0)

        def stage1(x_raw, t, st2_all):
            """LN stats on the f32 tile, then center+cast on ACT as (mean - x):
            global sign flip cancelled by negating w_out on the host."""
            st6 = sp.tile([P, 6], f32, tag="st6")
            nc.vector.bn_stats(st6[:], x_raw[:, t, :])
            nc.vector.bn_aggr(st2_all[:, t, :], st6[:])
            xc = xcp.tile([P, D], fp16, tag="xc")
            nc.scalar.activation(xc[:], x_raw[:, t, :], ACTF.Identity,
                                 bias=st2_all[:, t, 0:1], scale=-1.0)
            return xc

        def stage2(xc, t, f_dst, head_st6, pe_transpose, evac_scale=None):
            """Transpose (PE or DMA), 4-matmul projection, evac (optionally
            scaled), grouped per-head bn_stats."""
            slab = slp.tile([P, 4, P], fp16, tag="slab")
            if pe_transpose:
                for c in range(4):
                    pt = psT.tile([P, P], fp16, tag="ptx")
                    nc.tensor.transpose(pt[:], xc[:, c * P:(c + 1) * P], ident16[:])
                    if c % 2 == 0:
                        nc.scalar.copy(slab[:, c, :], pt[:])
                    else:
                        nc.vector.tensor_copy(slab[:, c, :], pt[:])
            else:
                nc.sync.dma_start_transpose(slab[:], xc[:])

            psf = psF.tile([P, D], f32, tag="pf")
            for c in range(4):
                nc.tensor.matmul(psf[:], slab[:, c, :], wgT_sb[:, c, :],
                                 start=(c == 0), stop=(c == 3))
            if evac_scale is not None:
                nc.scalar.activation(f_dst[:, t, :], psf[:], ACTF.Copy,
                                     scale=evac_scale)
            else:
                nc.scalar.copy(f_dst[:, t, :], psf[:])
            if head_st6 is not None:
                hsum, hsq = head_st6
                with nc.allow_low_precision(reason="head sums fit fp16"):
                    nc.vector.reduce_sum(
                        hsum[:, t, :],
                        f_dst[:, t, :].rearrange("p (h d) -> p h d", h=H),
                        axis=AXX)
                    sq = evp.tile([P, D], fp16, tag="sq")
                    nc.scalar.activation(sq[:], f_dst[:, t, :], ACTF.Square)
                    nc.vector.reduce_sum(
                        hsq[:, t, :],
                        sq[:].rearrange("p (h d) -> p h d", h=H), axis=AXX)

        # ---------------- k/v phase ----------------
        for t in range(KT):
            xk_c = stage1(xk_raw, t, st2_k)
            xv_c = stage1(xv_raw, t, st2_v)
            inv_sv_t = sp.tile([P, 1], f32, tag="invsv")
            nc.scalar.activation(inv_sv_t[:], st2_v[:, t, 1:2],
                                 ACTF.Abs_reciprocal_sqrt, bias=eps_b[:])
            stage2(xv_c, t, fv_all, None, False, evac_scale=inv_sv_t[:])
            stage2(xk_c, t, fk_all, (ksum, ksq), True)

        def head_derivs(hsum, hsq, st2var, nt, uniq, invn16, cmI16,
                        inv_s16, var16, inv_s32=None):
            """Batched per-(tile,head) scalars from the fp16 head sums.
            invn16 = rsqrt(sumsq);
            var16 = (sumsq - sum^2/64)/63 * inv_s^2 (unbiased, LN-unscaled);
            cmI16 = (sum/64) * inv_s (for the fused U slot1 build);
            inv_s = 1/sqrt(ln_var + eps)."""
            if inv_s32 is None:
                inv_s32 = sp.tile([P, nt], f32, tag="hd_invs",
                                  name=f"is{uniq}")[:]
            inv_s = inv_s32
            nc.scalar.activation(inv_s, st2var,
                                 ACTF.Abs_reciprocal_sqrt, bias=eps_b[:])
            with nc.allow_low_precision(reason="fp16 ample for tol 2e-2"):
                nc.scalar.activation(invn16[:], hsq, ACTF.Abs_reciprocal_sqrt)
                nc.vector.tensor_copy(inv_s16[:], inv_s)
                nc.vector.scalar_tensor_tensor(
                    cmI16[:], hsum, 1.0 / HD,
                    inv_s.unsqueeze(2).broadcast_to((P, nt, H)),
                    op0=ALU.mult, op1=ALU.mult)
                s2 = sp.tile([P, nt, H], f32, tag="hd_s2", name=f"a{uniq}")
                nc.vector.tensor_tensor(s2[:], hsum, hsum, op=ALU.mult)
                nc.vector.scalar_tensor_tensor(s2[:], s2[:], -1.0 / HD, hsq,
                                               op0=ALU.mult, op1=ALU.add)
                is2 = sp.tile([P, nt], f32, tag="hd_is2", name=f"v{uniq}")
                nc.vector.tensor_tensor(is2[:], inv_s, inv_s, op=ALU.mult)
                nc.vector.tensor_scalar_mul(s2[:], s2[:], 1.0 / (HD - 1))
                nc.vector.tensor_tensor(
                    var16[:], s2[:],
                    is2[:].unsqueeze(2).broadcast_to((P, nt, H)), op=ALU.mult)

        # ---- batched k derivations ----
        invn_k16 = cp.tile([P, KT, H], fp16)
        cmkI16 = cp.tile([P, KT, H], fp16)
        inv_sk16 = cp.tile([P, KT], fp16)
        kv16 = cp.tile([P, KT, H], fp16)
        head_derivs(ksum[:], ksq[:], st2_k[:, :, 1], KT, "k",
                    invn_k16[:], cmkI16[:], inv_sk16[:], kv16[:])

        # ---- batched U_k build (pure fp16) ----
        fk_v = fk_all[:].rearrange("p t (h d) -> p t h d", h=H)
        with nc.allow_low_precision(reason="fp16 ample for tol 2e-2"):
            nc.vector.tensor_tensor(
                uk_all[:, :, :, 0, :], fk_v,
                invn_k16[:].unsqueeze(3).broadcast_to((P, KT, H, HD)),
                op=ALU.mult)
            nc.vector.tensor_tensor(
                uk_all[:, :, :, 1, :], fk_v,
                inv_sk16[:].unsqueeze(2).unsqueeze(3).broadcast_to(
                    (P, KT, H, HD)), op=ALU.mult)
            nc.vector.tensor_tensor(
                uk_all[:, :, :, 1, :], uk_all[:, :, :, 1, :],
                cmkI16[:].unsqueeze(3).broadcast_to((P, KT, H, HD)),
                op=ALU.subtract)

        # ---- transposed per-head summary matrices ----
        # psmT[(h%2)*64+f, (h//2)*128+u] = sum_tok fv[tok,h*64+f]*U_k[tok,h,u]
        psmT = psM.tile([P, 512], f32, tag="pm")
        for h in range(H):
            po, co = HD * (h % 2), P * (h // 2)
            for t in range(KT):
                nc.tensor.matmul(
                    psmT[po:po + HD, co:co + P],
                    fv_all[:, t, h * HD:(h + 1) * HD],
                    uk_all[:, t, h, :, :].rearrange("p two d -> p (two d)"),
                    start=(t == 0), stop=(t == KT - 1))
        psm3 = psR.tile([P, 512], f32, tag="pr")
        for t in range(KT):
            nc.tensor.matmul(psm3[0:H, :], kv16[:, t, :], fv_all[:, t, :],
                             start=(t == 0), stop=(t == KT - 1))

        # BT_part: scale M1T cols by cos_w, M2T cols by c_cov
        BT_part = cp.tile([P, 512], fp16)
        btv = BT_part[:].rearrange("p (c u) -> p c u", c=4)
        pmv = psmT[:].rearrange("p (c u) -> p c u", c=4)
        nc.scalar.activation(btv[:, :, 0:HD], pmv[:, :, 0:HD], ACTF.Copy,
                             scale=cos_w)
        nc.scalar.activation(btv[:, :, HD:P], pmv[:, :, HD:P], ACTF.Copy,
                             scale=c_cov)
        # RW = (var_w/d) * blockdiag(m3) @ woT
        R_part = cp.tile([H, 512], fp16)
        nc.vector.scalar_tensor_tensor(R_part[:], psm3[0:H, :], c_var,
                                       bdmask[:], op0=ALU.mult, op1=ALU.mult)
        RT_sb = cp.tile([P, 4, H], fp16)
        for c in range(4):
            pt = psT.tile([P, P], fp16, tag="ptx")
            nc.tensor.transpose(pt[0:P, 0:H], R_part[:, c * P:(c + 1) * P],
                                ident16[0:H, 0:H])
            nc.scalar.copy(RT_sb[:, c, :], pt[0:P, 0:H])
        psrw = psR.tile([P, 512], f32, tag="pr")
        for c in range(4):
            nc.tensor.matmul(psrw[0:H, :], RT_sb[:, c, :], woT_sb[:, c, :],
                             start=(c == 0), stop=(c == 3))
        RW_part = cp.tile([H, 512], fp16)
        nc.scalar.copy(RW_part[:], psrw[0:H, :])

        # ---- pairwise AllReduce of [B^T; RW] (issued as early as possible;
        # the whole q-side pipeline below hides it) ----
        cc_in = nc.dram_tensor("cc_in", [P + H, 512], fp16)
        cc_out = nc.dram_tensor("cc_out", [P + H, 512], fp16)
        nc.sync.dma_start(cc_in[0:P, :], BT_part[:])
        nc.sync.dma_start(cc_in[P:P + H, :], RW_part[:])
        nc.gpsimd.collective_compute(
            "AllReduce", ALU.add,
            ins=[cc_in[:]], outs=[cc_out[:]],
            replica_groups=[[0, 1], [2, 3], [4, 5], [6, 7]])

        # ---------------- q phase (hides the collective) ----------------
        invn_q16 = cp.tile([P, QT, H], fp16)
        cmqI16 = cp.tile([P, QT, H], fp16)   # computed but unused for q
        inv_sq16 = cp.tile([P, QT], fp16)
        inv_sq32 = cp.tile([P, QT], f32)
        qv16 = cp.tile([P, QT, H], fp16)

        def uq_tile(t):
            fq_v3 = fq_all[:, t, :].rearrange("p (h d) -> p h d", h=H)
            with nc.allow_low_precision(reason="fp16 ample for tol 2e-2"):
                nc.vector.tensor_tensor(
                    uq_all[:, t, 0:H, 0, :], fq_v3,
                    invn_q16[:, t, :].unsqueeze(2).broadcast_to((P, H, HD)),
                    op=ALU.mult)
                nc.vector.tensor_scalar_mul(
                    uq_all[:, t, 0:H, 1, :], fq_v3, inv_sq32[:, t:t + 1])
                nc.vector.tensor_copy(uq_all[:, t, 8, 0, 0:H], qv16[:, t, :])
            uqT = uqp.tile([P, 9, P], fp16, tag="uqT", name=f"uqT{t}")
            nc.sync.dma_start_transpose(
                uqT[:],
                uq_all[:, t, :, :, :].rearrange("p n two d -> p (n two d)"))
            return uqT

        HF = QT // 2
        uqTs = {}
        for t in range(HF):
            stage2(stage1(xq_raw, t, st2_q), t, fq_all, (qsum, qsq), True)
        head_derivs(qsum[:, 0:HF, :], qsq[:, 0:HF, :], st2_q[:, 0:HF, 1],
                    HF, "qa", invn_q16[:, 0:HF, :], cmqI16[:, 0:HF, :],
                    inv_sq16[:, 0:HF], qv16[:, 0:HF, :],
                    inv_s32=inv_sq32[:, 0:HF])
        for t in range(HF, QT):
            stage2(stage1(xq_raw, t, st2_q), t, fq_all, (qsum, qsq), True)
            uqTs[t - HF] = uq_tile(t - HF)
        head_derivs(qsum[:, HF:QT, :], qsq[:, HF:QT, :], st2_q[:, HF:QT, 1],
                    QT - HF, "qb", invn_q16[:, HF:QT, :], cmqI16[:, HF:QT, :],
                    inv_sq16[:, HF:QT], qv16[:, HF:QT, :],
                    inv_s32=inv_sq32[:, HF:QT])
        for t in range(HF, QT):
            uqTs[t] = uq_tile(t)

        # ---- C build: fold w_out into the per-head summaries ----
        BT_sb = cp.tile([P, 4, P], fp16)
        nc.sync.dma_start(
            BT_sb[:], cc_out[0:P, :].rearrange("p (c u) -> p c u", c=4))
        RW_sb = cp.tile([H, 512], fp16)
        nc.sync.dma_start(RW_sb[:], cc_out[P:P + H, :])
        for h in range(H):
            po = HD * (h % 2)
            psc = psF.tile([P, D], f32, tag="pf")
            nc.tensor.matmul(psc[:], BT_sb[po:po + HD, h // 2, :],
                             woT_sb[po:po + HD, h // 2, :],
                             start=True, stop=True)
            if h % 2 == 0:
                nc.scalar.copy(C_sb[:, h, :], psc[:])
            else:
                nc.vector.tensor_copy(C_sb[:, h, :], psc[:])
        nc.vector.tensor_copy(C_sb[0:H, 8, :], RW_sb[:])

        # ---- attention + output projection (9 fused matmuls per tile) ----
        for t in range(QT):
            pso = psF.tile([P, D], f32, tag="pf")
            for h in range(9):
                nc.tensor.matmul(pso[:], uqTs[t][:, h, :], C_sb[:, h, :],
                                 start=(h == 0), stop=(h == 8))
            o_sb = evp.tile([P, D], f32, tag="o_sb")
            if t % 2 == 0:
                nc.vector.tensor_copy(o_sb[:], pso[:])
            else:
                nc.scalar.copy(o_sb[:], pso[:])
            nc.sync.dma_start(out_d[t * P:(t + 1) * P, :], o_sb[:])

    nc.compile()
    return nc


_NC_CACHE = {}


def kernel(q, k, v, ln_gamma, ln_beta, w_in, w_out, b_out, cov_w_raw, var_w_raw):
    q = np.ascontiguousarray(np.asarray(q, dtype=np.float32))
    k = np.ascontiguousarray(np.asarray(k, dtype=np.float32))
    v = np.ascontiguousarray(np.asarray(v, dtype=np.float32))
    ln_gamma = np.asarray(ln_gamma, dtype=np.float32)
    ln_beta = np.asarray(ln_beta, dtype=np.float32)
    w_in = np.asarray(w_in, dtype=np.float32)
    w_out = np.asarray(w_out, dtype=np.float32)
    b_out = np.asarray(b_out, dtype=np.float32)
    assert np.all(ln_beta == 0.0), "kernel assumes LayerNorm beta == 0"
    assert np.all(b_out == 0.0), "kernel assumes b_out == 0"

    def sigmoid(x):
        return 1.0 / (1.0 + np.exp(-float(x)))

    cov_w = sigmoid(cov_w_raw)
    var_w = sigmoid(var_w_raw)
    cos_w = 1.0 - cov_w - var_w

    wg = w_in * ln_gamma[None, :]          # [inner, d]
    wgT = np.ascontiguousarray(wg.T)       # [d, inner]
    woT = np.ascontiguousarray(-w_out.T)   # negated: cancels the (mean-x) flip

    key = (round(float(cos_w), 8), round(float(cov_w), 8), round(float(var_w), 8))
    if key not in _NC_CACHE:
        _NC_CACHE[key] = build_kernel(cos_w, cov_w, var_w)
    nc = _NC_CACHE[key]

    in_maps = []
    for c in range(NCORES):
        g, s = c // 2, c % 2
        in_maps.append({
            "xq": np.ascontiguousarray(q[g, s * TQ:(s + 1) * TQ, :]),
            "xk": np.ascontiguousarray(k[g, s * TK:(s + 1) * TK, :]),
            "xv": np.ascontiguousarray(v[g, s * TK:(s + 1) * TK, :]),
            "wgT": wgT,
            "woT": woT,
        })
    res = run_bass_kernel_spmd(nc, in_maps, core_ids=list(range(NCORES))).results

    out = np.empty((QG, N, D), dtype=np.float32)
    for c in range(NCORES):
        g, s = c // 2, c % 2
        out[g, s * TQ:(s + 1) * TQ, :] = res[c]["out"]
    return out


# revision 27
# speedup vs baseline: 1.0992x; 1.0992x over previous
"""Trainium2 Bass kernel for nn_Attention_69801808495308 (v2).

Softmax-free attention: attn = cos_w*cossim + cov_w*cov/d + var_w*varprod/d is
linear in k-side summaries, so attn @ f_v reassociates into per-head 64x64
matrices (linear-attention trick) - no NxN score matrix is materialized.

Per (group g, head h), with fk/fv/fq the projected features:
  M1 = (fk/||fk||)^T fv_true        [64,64]
  M2 = (fk - mean(fk))^T fv_true    [64,64]   (columns sum to 0 -> q-centering free)
  m3 = kvar^T fv_true               [64]
  out = sum_h U_q[h] @ C[h] + qvar @ C[8]
  where C[h] = B_h^T-derived [128,512] folds the output projection into the
  per-head summary (B_h = [cos_w*M1; (cov_w/d)*M2]) and C[8] rows 0..7 hold
  RW = (var_w/d)*blockdiag(m3) @ woT - the qvar term rides as a 9th "head".

Sharding: 8 cores = (group g in 0..3) x (row half s in 0..1); q and k/v rows
split across the pair. The per-core partial [B^T; RW] (fp16, 139KB) is
finished by a pairwise AllReduce - the only cross-core communication.

v2 schedule (vs v1): all k/v work first -> M-chain -> AllReduce issued as
early as possible -> the whole q-side pipeline (projection, U_q builds, DMA
transposes) hides the collective -> C build -> attention tail starts with a
warm PE (v1 lost ~50us to a post-M dead zone + HAM cold-clock on the tail).

Other v2 changes:
- M-chain emitted transposed (lhsT=fv, rhs=U_k) so B^T is produced directly;
  C_h = matmul(lhsT=B_h^T, rhs=woT rows) needs no B transposes.
- Head sums/sumsq via one grouped bn_stats per tile ([P,8,64]->[P,8,6])
  instead of reduce_sum+square+reduce_sum (saves ~1.5us DVE per tile).
- x tiles loaded f32 via HWDGE (big 1MB DMAs); the f32->fp16 cast happens in
  the LN-centering ACT op. GpSimd stays idle so the collective trigger is
  not queued behind SWDGE work.
- U-tensor builds in pure fp16 (per-token scalars pre-cast) for 2x DVE rate.
- LayerNorm folded: centering as (mean - x); global sign flip cancelled by
  negating w_out on host (beta, b_out asserted 0). 1/sigma absorbed into
  U-tensor builds; cosine term is scale-invariant.
"""
import numpy as np
from contextlib import ExitStack

import concourse.bass as bass
from concourse import bacc
import concourse.tile as tile
import concourse.mybir as mybir
from concourse.bass_utils import run_bass_kernel_spmd
from concourse.masks import make_identity

f32 = mybir.dt.float32
fp16 = mybir.dt.float16
ALU = mybir.AluOpType
ACTF = mybir.ActivationFunctionType
AXX = mybir.AxisListType.X

QG, N, D = 4, 2048, 512
H, HD = 8, 64
P = 128
LN_EPS = 1e-5
TQ, TK = N // 2, N // 2
QT, KT = TQ // P, TK // P
NCORES = 8


def build_kernel(cos_w, cov_w, var_w):
    c_cov = cov_w / HD
    c_var = var_w / HD

    nc = bacc.Bacc("TRN2", target_bir_lowering=False, debug=False,
                   num_devices=NCORES)
    # 3D views of the row-major [rows, D] buffers: partition p holds rows
    # p*KT..p*KT+KT-1, so every DMA descriptor is a contiguous 2-16KB run.
    # The token->(partition,tile) assignment is arbitrary as long as loads,
    # stores and the host reshape agree (attention mixes tokens linearly).
    xq = nc.declare_dram_parameter("xq", [P, QT, D], f32, isOutput=False)
    xk = nc.declare_dram_parameter("xk", [P, KT, D], f32, isOutput=False)
    xv = nc.declare_dram_parameter("xv", [P, KT, D], f32, isOutput=False)
    wgT_d = nc.declare_dram_parameter("wgT", [D, D], f32, isOutput=False)
    woT_d = nc.declare_dram_parameter("woT", [D, D], f32, isOutput=False)
    out_d = nc.declare_dram_parameter("out", [P, QT, D], f32, isOutput=True)

    with tile.TileContext(nc) as tc, ExitStack() as ctx:
        cp = ctx.enter_context(tc.tile_pool(name="cp", bufs=1))
        xcp = ctx.enter_context(tc.tile_pool(name="xcp", bufs=4))
        slp = ctx.enter_context(tc.tile_pool(name="slp", bufs=4))
        sp = ctx.enter_context(tc.tile_pool(name="sp", bufs=4))
        uqp = ctx.enter_context(tc.tile_pool(name="uqp", bufs=1))
        evp = ctx.enter_context(tc.tile_pool(name="evp", bufs=3))
        psF = ctx.enter_context(tc.tile_pool(name="psF", bufs=3, space="PSUM"))
        psT = ctx.enter_context(tc.tile_pool(name="psT", bufs=3, space="PSUM"))
        psM = ctx.enter_context(tc.tile_pool(name="psM", bufs=1, space="PSUM"))
        psR = ctx.enter_context(tc.tile_pool(name="psR", bufs=1, space="PSUM"))

        # ---- constants first (identity unblocks the PE transposes) ----
        ident16 = cp.tile([P, P], fp16)
        make_identity(nc, ident16)

        # ---- SWDGE cast-loads (f32->fp16 during DMA; contiguous 2-tile
        # chunks per partition, k/v interleaved) ----
        xk_raw = cp.tile([P, KT, D], fp16)
        xv_raw = cp.tile([P, KT, D], fp16)
        xq_raw = cp.tile([P, QT, D], fp16)
        for c in range(0, KT, 2):
            nc.gpsimd.dma_start(xk_raw[:, c:c + 2, :], xk[:, c:c + 2, :])
            nc.gpsimd.dma_start(xv_raw[:, c:c + 2, :], xv[:, c:c + 2, :])
        wgT_sb = cp.tile([P, 4, D], fp16)
        nc.gpsimd.dma_start(wgT_sb[:], wgT_d[:].rearrange("(c p) n -> p c n", p=P))
        for c in range(0, QT, 4):
            nc.gpsimd.dma_start(xq_raw[:, c:c + 4, :], xq[:, c:c + 4, :])
        woT_sb = cp.tile([P, 4, D], fp16)
        nc.gpsimd.dma_start(woT_sb[:], woT_d[:].rearrange("(c p) n -> p c n", p=P))
        eps_b = cp.tile([P, 1], f32)
        nc.vector.memset(eps_b[:], LN_EPS)
        bdmask = cp.tile([H, 512], f32)
        nc.gpsimd.memset(bdmask[:], 0.0)
        nc.gpsimd.affine_select(
            out=bdmask[:].rearrange("p (b d) -> p b d", b=H),
            in_=bdmask[:].rearrange("p (b d) -> p b d", b=H),
            compare_op=ALU.not_equal, fill=1.0, base=0,
            pattern=[[-1, H], [0, HD]], channel_multiplier=1)

        # ---- persistent state ----
        fk_all = cp.tile([P, KT, D], fp16)
        fv_all = cp.tile([P, KT, D], fp16)
        fq_all = cp.tile([P, QT, D], fp16)
        uk_all = cp.tile([P, KT, H, 2, HD], fp16)
        uq_all = cp.tile([P, QT, 9, 2, HD], fp16)   # block 8 = qvar row + zeros
        st2_k = cp.tile([P, KT, 2], f32)
        st2_v = cp.tile([P, KT, 2], f32)
        st2_q = cp.tile([P, QT, 2], f32)
        ksum = cp.tile([P, KT, H], fp16)
        ksq = cp.tile([P, KT, H], fp16)
        qsum = cp.tile([P, QT, H], fp16)
        qsq = cp.tile([P, QT, H], fp16)
        C_sb = cp.tile([P, 9, D], fp16)
        nc.gpsimd.memset(uq_all[:, :, 8, :, :], 0.0)
        nc.gpsimd.memset(C_sb[:, 8, :], 0.0)

        def stage1(x_raw, t, st2_all):
            """LN stats on the f32 tile, then center+cast on ACT as (mean - x):
            global sign flip cancelled by negating w_out on the host."""
            st6 = sp.tile([P, 6], f32, tag="st6")
            nc.vector.bn_stats(st6[:], x_raw[:, t, :])
            nc.vector.bn_aggr(st2_all[:, t, :], st6[:])
            xc = xcp.tile([P, D], fp16, tag="xc")
            nc.scalar.activation(xc[:], x_raw[:, t, :], ACTF.Identity,
                                 bias=st2_all[:, t, 0:1], scale=-1.0)
            return xc

        def stage2(xc, t, f_dst, head_st6, pe_transpose, evac_scale=None):
            """Transpose (PE or DMA), 4-matmul projection, evac (optionally
            scaled), grouped per-head bn_stats."""
            slab = slp.tile([P, 4, P], fp16, tag="slab")
            if pe_transpose:
                for c in range(4):
                    pt = psT.tile([P, P], fp16, tag="ptx")
                    nc.tensor.transpose(pt[:], xc[:, c * P:(c + 1) * P], ident16[:])
                    if c % 2 == 0:
                        nc.scalar.copy(slab[:, c, :], pt[:])
                    else:
                        nc.vector.tensor_copy(slab[:, c, :], pt[:])
            else:
                nc.sync.dma_start_transpose(slab[:], xc[:])

            psf = psF.tile([P, D], f32, tag="pf")
            for c in range(4):
                nc.tensor.matmul(psf[:], slab[:, c, :], wgT_sb[:, c, :],
                                 start=(c == 0), stop=(c == 3))
            if evac_scale is not None:
                nc.scalar.activation(f_dst[:, t, :], psf[:], ACTF.Copy,
                                     scale=evac_scale)
            else:
                nc.scalar.copy(f_dst[:, t, :], psf[:])
            if head_st6 is not None:
                hsum, hsq = head_st6
                with nc.allow_low_precision(reason="head sums fit fp16"):
                    nc.vector.reduce_sum(
                        hsum[:, t, :],
                        f_dst[:, t, :].rearrange("p (h d) -> p h d", h=H),
                        axis=AXX)
                    sq = evp.tile([P, D], fp16, tag="sq")
                    nc.gpsimd.tensor_mul(sq[:], f_dst[:, t, :], f_dst[:, t, :])
                    nc.vector.reduce_sum(
                        hsq[:, t, :],
                        sq[:].rearrange("p (h d) -> p h d", h=H), axis=AXX)

        # ---------------- k/v phase ----------------
        for t in range(KT):
            xk_c = stage1(xk_raw, t, st2_k)
            xv_c = stage1(xv_raw, t, st2_v)
            inv_sv_t = sp.tile([P, 1], f32, tag="invsv")
            nc.scalar.activation(inv_sv_t[:], st2_v[:, t, 1:2],
                                 ACTF.Abs_reciprocal_sqrt, bias=eps_b[:])
            stage2(xv_c, t, fv_all, None, False, evac_scale=inv_sv_t[:])
            stage2(xk_c, t, fk_all, (ksum, ksq), True)

        def head_derivs(hsum, hsq, st2var, nt, uniq, invn16, cmI16,
                        inv_s16, var16, inv_s32=None):
            """Batched per-(tile,head) scalars from the fp16 head sums.
            invn16 = rsqrt(sumsq);
            var16 = (sumsq - sum^2/64)/63 * inv_s^2 (unbiased, LN-unscaled);
            cmI16 = (sum/64) * inv_s (for the fused U slot1 build);
            inv_s = 1/sqrt(ln_var + eps)."""
            if inv_s32 is None:
                inv_s32 = sp.tile([P, nt], f32, tag="hd_invs",
                                  name=f"is{uniq}")[:]
            inv_s = inv_s32
            nc.scalar.activation(inv_s, st2var,
                                 ACTF.Abs_reciprocal_sqrt, bias=eps_b[:])
            with nc.allow_low_precision(reason="fp16 ample for tol 2e-2"):
                nc.scalar.activation(invn16[:], hsq, ACTF.Abs_reciprocal_sqrt)
                nc.vector.tensor_copy(inv_s16[:], inv_s)
                nc.vector.scalar_tensor_tensor(
                    cmI16[:], hsum, 1.0 / HD,
                    inv_s.unsqueeze(2).broadcast_to((P, nt, H)),
                    op0=ALU.mult, op1=ALU.mult)
                s2 = sp.tile([P, nt, H], f32, tag="hd_s2", name=f"a{uniq}")
                nc.vector.tensor_tensor(s2[:], hsum, hsum, op=ALU.mult)
                nc.vector.scalar_tensor_tensor(s2[:], s2[:], -1.0 / HD, hsq,
                                               op0=ALU.mult, op1=ALU.add)
                is2 = sp.tile([P, nt], f32, tag="hd_is2", name=f"v{uniq}")
                nc.vector.tensor_tensor(is2[:], inv_s, inv_s, op=ALU.mult)
                nc.vector.tensor_scalar_mul(s2[:], s2[:], 1.0 / (HD - 1))
                nc.vector.tensor_tensor(
                    var16[:], s2[:],
                    is2[:].unsqueeze(2).broadcast_to((P, nt, H)), op=ALU.mult)

        # ---- batched k derivations ----
        invn_k16 = cp.tile([P, KT, H], fp16)
        cmkI16 = cp.tile([P, KT, H], fp16)
        inv_sk16 = cp.tile([P, KT], fp16)
        kv16 = cp.tile([P, KT, H], fp16)
        head_derivs(ksum[:], ksq[:], st2_k[:, :, 1], KT, "k",
                    invn_k16[:], cmkI16[:], inv_sk16[:], kv16[:])

        # ---- batched U_k build (pure fp16) ----
        fk_v = fk_all[:].rearrange("p t (h d) -> p t h d", h=H)
        with nc.allow_low_precision(reason="fp16 ample for tol 2e-2"):
            nc.vector.tensor_tensor(
                uk_all[:, :, :, 0, :], fk_v,
                invn_k16[:].unsqueeze(3).broadcast_to((P, KT, H, HD)),
                op=ALU.mult)
            nc.vector.tensor_tensor(
                uk_all[:, :, :, 1, :], fk_v,
                inv_sk16[:].unsqueeze(2).unsqueeze(3).broadcast_to(
                    (P, KT, H, HD)), op=ALU.mult)
            nc.vector.tensor_tensor(
                uk_all[:, :, :, 1, :], uk_all[:, :, :, 1, :],
                cmkI16[:].unsqueeze(3).broadcast_to((P, KT, H, HD)),
                op=ALU.subtract)

        # ---- transposed per-head summary matrices ----
        # psmT[(h%2)*64+f, (h//2)*128+u] = sum_tok fv[tok,h*64+f]*U_k[tok,h,u]
        psmT = psM.tile([P, 512], f32, tag="pm")
        for h in range(H):
            po, co = HD * (h % 2), P * (h // 2)
            for t in range(KT):
                nc.tensor.matmul(
                    psmT[po:po + HD, co:co + P],
                    fv_all[:, t, h * HD:(h + 1) * HD],
                    uk_all[:, t, h, :, :].rearrange("p two d -> p (two d)"),
                    start=(t == 0), stop=(t == KT - 1))
        psm3 = psR.tile([P, 512], f32, tag="pr")
        for t in range(KT):
            nc.tensor.matmul(psm3[0:H, :], kv16[:, t, :], fv_all[:, t, :],
                             start=(t == 0), stop=(t == KT - 1))

        # BT_part: scale M1T cols by cos_w, M2T cols by c_cov
        BT_part = cp.tile([P, 512], fp16)
        btv = BT_part[:].rearrange("p (c u) -> p c u", c=4)
        pmv = psmT[:].rearrange("p (c u) -> p c u", c=4)
        nc.scalar.activation(btv[:, :, 0:HD], pmv[:, :, 0:HD], ACTF.Copy,
                             scale=cos_w)
        nc.scalar.activation(btv[:, :, HD:P], pmv[:, :, HD:P], ACTF.Copy,
                             scale=c_cov)
        # RW = (var_w/d) * blockdiag(m3) @ woT
        R_part = cp.tile([H, 512], fp16)
        nc.vector.scalar_tensor_tensor(R_part[:], psm3[0:H, :], c_var,
                                       bdmask[:], op0=ALU.mult, op1=ALU.mult)
        RT_sb = cp.tile([P, 4, H], fp16)
        for c in range(4):
            pt = psT.tile([P, P], fp16, tag="ptx")
            nc.tensor.transpose(pt[0:P, 0:H], R_part[:, c * P:(c + 1) * P],
                                ident16[0:H, 0:H])
            nc.scalar.copy(RT_sb[:, c, :], pt[0:P, 0:H])
        psrw = psR.tile([P, 512], f32, tag="pr")
        for c in range(4):
            nc.tensor.matmul(psrw[0:H, :], RT_sb[:, c, :], woT_sb[:, c, :],
                             start=(c == 0), stop=(c == 3))
        RW_part = cp.tile([H, 512], fp16)
        nc.scalar.copy(RW_part[:], psrw[0:H, :])

        # ---- pairwise AllReduce of [B^T; RW] (issued as early as possible;
        # the whole q-side pipeline below hides it) ----
        cc_in = nc.dram_tensor("cc_in", [P + H, 512], fp16)
        cc_out = nc.dram_tensor("cc_out", [P + H, 512], fp16)
        nc.sync.dma_start(cc_in[0:P, :], BT_part[:])
        nc.sync.dma_start(cc_in[P:P + H, :], RW_part[:])
        nc.gpsimd.collective_compute(
            "AllReduce", ALU.add,
            ins=[cc_in[:]], outs=[cc_out[:]],
            replica_groups=[[0, 1], [2, 3], [4, 5], [6, 7]])

        # ---------------- q phase (hides the collective) ----------------
        invn_q16 = cp.tile([P, QT, H], fp16)
        cmqI16 = cp.tile([P, QT, H], fp16)   # computed but unused for q
        inv_sq16 = cp.tile([P, QT], fp16)
        inv_sq32 = cp.tile([P, QT], f32)
        qv16 = cp.tile([P, QT, H], fp16)

        def uq_tile(t):
            fq_v3 = fq_all[:, t, :].rearrange("p (h d) -> p h d", h=H)
            with nc.allow_low_precision(reason="fp16 ample for tol 2e-2"):
                nc.vector.tensor_tensor(
                    uq_all[:, t, 0:H, 0, :], fq_v3,
                    invn_q16[:, t, :].unsqueeze(2).broadcast_to((P, H, HD)),
                    op=ALU.mult)
                nc.vector.tensor_scalar_mul(
                    uq_all[:, t, 0:H, 1, :], fq_v3, inv_sq32[:, t:t + 1])
                nc.vector.tensor_copy(uq_all[:, t, 8, 0, 0:H], qv16[:, t, :])
            uqT = uqp.tile([P, 9, P], fp16, tag="uqT", name=f"uqT{t}")
            nc.sync.dma_start_transpose(
                uqT[:],
                uq_all[:, t, :, :, :].rearrange("p n two d -> p (n two d)"))
            return uqT

        HF = QT // 2
        uqTs = {}
        for t in range(HF):
            stage2(stage1(xq_raw, t, st2_q), t, fq_all, (qsum, qsq), True)
        head_derivs(qsum[:, 0:HF, :], qsq[:, 0:HF, :], st2_q[:, 0:HF, 1],
                    HF, "qa", invn_q16[:, 0:HF, :], cmqI16[:, 0:HF, :],
                    inv_sq16[:, 0:HF], qv16[:, 0:HF, :],
                    inv_s32=inv_sq32[:, 0:HF])
        for t in range(HF, QT):
            stage2(stage1(xq_raw, t, st2_q), t, fq_all, (qsum, qsq), True)
            uqTs[t - HF] = uq_tile(t - HF)
        head_derivs(qsum[:, HF:QT, :], qsq[:, HF:QT, :], st2_q[:, HF:QT, 1],
                    QT - HF, "qb", invn_q16[:, HF:QT, :], cmqI16[:, HF:QT, :],
                    inv_sq16[:, HF:QT], qv16[:, HF:QT, :],
                    inv_s32=inv_sq32[:, HF:QT])
        for t in range(HF, QT):
            uqTs[t] = uq_tile(t)

        # ---- C build: fold w_out into the per-head summaries ----
        BT_sb = cp.tile([P, 4, P], fp16)
        nc.sync.dma_start(
            BT_sb[:], cc_out[0:P, :].rearrange("p (c u) -> p c u", c=4))
        RW_sb = cp.tile([H, 512], fp16)
        nc.sync.dma_start(RW_sb[:], cc_out[P:P + H, :])
        for h in range(H):
            po = HD * (h % 2)
            psc = psF.tile([P, D], f32, tag="pf")
            nc.tensor.matmul(psc[:], BT_sb[po:po + HD, h // 2, :],
                             woT_sb[po:po + HD, h // 2, :],
                             start=True, stop=True)
            if h % 2 == 0:
                nc.scalar.copy(C_sb[:, h, :], psc[:])
            else:
                nc.vector.tensor_copy(C_sb[:, h, :], psc[:])
        nc.vector.tensor_copy(C_sb[0:H, 8, :], RW_sb[:])

        # ---- attention + output projection (9 fused matmuls per tile) ----
        for t in range(QT):
            pso = psF.tile([P, D], f32, tag="pf")
            for h in range(9):
                nc.tensor.matmul(pso[:], uqTs[t][:, h, :], C_sb[:, h, :],
                                 start=(h == 0), stop=(h == 8))
            o_sb = evp.tile([P, D], f32, tag="o_sb")
            if t % 2 == 0:
                nc.vector.tensor_copy(o_sb[:], pso[:])
            else:
                nc.scalar.copy(o_sb[:], pso[:])
            nc.sync.dma_start(out_d[:, t, :], o_sb[:])

    nc.compile()
    return nc


_NC_CACHE = {}


def kernel(q, k, v, ln_gamma, ln_beta, w_in, w_out, b_out, cov_w_raw, var_w_raw):
    q = np.ascontiguousarray(np.asarray(q, dtype=np.float32))
    k = np.ascontiguousarray(np.asarray(k, dtype=np.float32))
    v = np.ascontiguousarray(np.asarray(v, dtype=np.float32))
    ln_gamma = np.asarray(ln_gamma, dtype=np.float32)
    ln_beta = np.asarray(ln_beta, dtype=np.float32)
    w_in = np.asarray(w_in, dtype=np.float32)
    w_out = np.asarray(w_out, dtype=np.float32)
    b_out = np.asarray(b_out, dtype=np.float32)
    assert np.all(ln_beta == 0.0), "kernel assumes LayerNorm beta == 0"
    assert np.all(b_out == 0.0), "kernel assumes b_out == 0"

    def sigmoid(x):
        return 1.0 / (1.0 + np.exp(-float(x)))

    cov_w = sigmoid(cov_w_raw)
    var_w = sigmoid(var_w_raw)
    cos_w = 1.0 - cov_w - var_w

    wg = w_in * ln_gamma[None, :]          # [inner, d]
    wgT = np.ascontiguousarray(wg.T)       # [d, inner]
    woT = np.ascontiguousarray(-w_out.T)   # negated: cancels the (mean-x) flip

    key = (round(float(cos_w), 8), round(float(cov_w), 8), round(float(var_w), 8))
    if key not in _NC_CACHE:
        _NC_CACHE[key] = build_kernel(cos_w, cov_w, var_w)
    nc = _NC_CACHE[key]

    in_maps = []
    for c in range(NCORES):
        g, s = c // 2, c % 2
        in_maps.append({
            "xq": np.ascontiguousarray(
                q[g, s * TQ:(s + 1) * TQ, :]).reshape(P, QT, D),
            "xk": np.ascontiguousarray(
                k[g, s * TK:(s + 1) * TK, :]).reshape(P, KT, D),
            "xv": np.ascontiguousarray(
                v[g, s * TK:(s + 1) * TK, :]).reshape(P, KT, D),
            "wgT": wgT,
            "woT": woT,
        })
    res = run_bass_kernel_spmd(nc, in_maps, core_ids=list(range(NCORES))).results

    out = np.empty((QG, N, D), dtype=np.float32)
    for c in range(NCORES):
        g, s = c // 2, c % 2
        out[g, s * TQ:(s + 1) * TQ, :] = res[c]["out"].reshape(TQ, D)
    return out
